# revision 46
# baseline (speedup 1.0000x reference)
"""Cross-modal attention kernel for Trainium2 -- data-parallel over batch on 8 cores.

Reference computation per sample (C=256, H=W=64, N=H*W=4096, dqk=32):
    q = Wq @ x + bq; k = Wk @ y + bk; v = Wv @ y + bv
    out = gamma * (v @ softmax_j(q^T k)^T) + x

The Activation engine is the roofline: exp() over the [N, N] energy matrix is
131072 columns at 0.8333 ns/col, plus a fixed ~185 ns access penalty per
activation instruction.  The schedule maximizes exp() instruction size
([128, 1536] = 3 PSUM banks, 11 instructions per i-block, ~127 us busy) and
keeps the engine streaming gap-free from ~6 us on:

  - Projections are FUSED into the first two i-blocks: all k/q sub-chunks
    run just ahead of block 0's energy matmuls, v during block 1.
  - q/k are built directly in the [16, 2, N] DoubleRow layout: each 256-col
    sub-chunk is four fp8 DoubleRow matmuls into a [16, 2, 256] PSUM tile and
    one 3D-AP DVE tensor_copy (f32 -> fp8).  The bias rides along as a 65th
    contraction row (x/y carry a ones-row, Wq/Wk carry the bias).
  - Inputs are uploaded pre-quantized fp8 (2 MB total): gamma is folded into
    Wv/bv so the softmax scale step disappears; the f32 residual x streams in
    during the steady state when DMA is idle.
  - Every matmul runs fp8 DoubleRow (0.5 cyc/row).  PE busy ~66 us.
  - PSUM (8 banks): et double-buffer 6 + one 2-bank ring that time-shares
    k/q projection tiles (block 0), v tiles (block 1), then den/AV:
    den and AV run as deferred 16-step DoubleRow bursts one block later
    (pt lives in a 3-deep SBUF ring), except the last block where den and
    AV-ch0 run eagerly in the freed ring slots so the post-exp drain is only
    the AV-ch1 burst plus one tail chain.
  - Softmax normalization happens on the [C, IBLK] output (reciprocal of the
    ones-matmul denominator), never on the [N, N] matrix.
"""

import sys

if "/opt/trn_rl_repo" not in sys.path:
    sys.path.insert(0, "/opt/trn_rl_repo")

import ml_dtypes
import numpy as np

import concourse.bacc as bacc
import concourse.mybir as mybir
import concourse.tile as tile
from concourse.bass_utils import run_bass_kernel_spmd

F32 = mybir.dt.float32
F8 = mybir.dt.float8e4
F8NP = ml_dtypes.float8_e4m3

B, C, HW, D = 8, 256, 4096, 32
CH = C // 128
IBLK = 512
NIB = HW // IBLK          # 8 i-blocks
NJT = HW // 128           # 32 j-tiles
NPAIR = NJT // 2          # 16 DoubleRow pairs
NGE = 11                  # energy groups per block: 10x3 + 1x2 j-tiles
EXPF = mybir.ActivationFunctionType.Exp
MULT = mybir.AluOpType.mult
ADD = mybir.AluOpType.add
DROW = mybir.MatmulPerfMode.DoubleRow


def _pair_hi(g):
    # highest den/av pair index ready after act group g of a block
    return min((3 * g + 1) // 2, NPAIR - 1)


def _build():
    nc = bacc.Bacc("TRN2", target_bir_lowering=False, debug=False, num_devices=8)

    x8d = nc.dram_tensor("x8d", [65, 4 * HW], F8, kind="ExternalInput")
    y8d = nc.dram_tensor("y8d", [65, 4 * HW], F8, kind="ExternalInput")
    xfd = nc.dram_tensor("xfd", [C, HW], F32, kind="ExternalInput")
    wq8d = nc.dram_tensor("wq8d", [65, 4 * D], F8, kind="ExternalInput")
    wk8d = nc.dram_tensor("wk8d", [65, 4 * D], F8, kind="ExternalInput")
    wv8d = nc.dram_tensor("wv8d", [64, 4 * C], F8, kind="ExternalInput")
    gbvd = nc.dram_tensor("gbvd", [128, CH], F32, kind="ExternalInput")
    out = nc.dram_tensor("out", [C, HW], F32, kind="ExternalOutput")

    tc = tile.TileContext(nc)
    with tc:
        with (
            tc.tile_pool(name="cst", bufs=1) as cst,
            tc.tile_pool(name="ptp", bufs=3) as ptp,
            tc.tile_pool(name="wrk", bufs=2) as wrk,
            tc.tile_pool(name="psE", bufs=1, space="PSUM") as psE,
        ):
            wq_sb = cst.tile([65, 4 * D], F8)
            wk_sb = cst.tile([65, 4 * D], F8)
            wv_sb = cst.tile([64, 4 * C], F8)
            gbv_sb = cst.tile([128, CH], F32)
            ones_sb = cst.tile([128, 2 * 128], F8)
            x_sb = cst.tile([65, 4 * HW], F8)
            y_sb = cst.tile([65, 4 * HW], F8)
            q4f = cst.tile([16, 2 * HW], F8)
            k4f = cst.tile([16, 2 * HW], F8)
            vt = cst.tile([128, NJT * C], F8)

            nc.vector.memset(ones_sb[:], 1.0)
            # small weights ride the SWDGE (gpsimd) queue, in parallel with
            # the HWDGE input stream; gbv isn't needed until block 2
            nc.gpsimd.dma_start(wk_sb[:], wk8d[:])
            nc.gpsimd.dma_start(wq_sb[:], wq8d[:])
            nc.gpsimd.dma_start(gbv_sb[:], gbvd[:])

            # y chunk 0 issues from the empty Activation HWDGE queue (its
            # descriptor goes out before SP's preamble), x chunk 0 leads SP
            y4 = y_sb[:].rearrange("P (b N) -> P b N", b=4)
            x4 = x_sb[:].rearrange("P (b N) -> P b N", b=4)
            nc.scalar.dma_start(
                y4[:, :, 0:IBLK],
                y8d[:, 0:4 * IBLK].rearrange("P (b N) -> P b N", b=4))
            nc.sync.dma_start(
                x4[:, :, 0:IBLK],
                x8d[:, 0:4 * IBLK].rearrange("P (b N) -> P b N", b=4))
            nc.sync.dma_start(wv_sb[:], wv8d[:])
            for g in range(1, NIB):
                nc.sync.dma_start(
                    y4[:, :, g * IBLK:(g + 1) * IBLK],
                    y8d[:, g * 4 * IBLK:(g + 1) * 4 * IBLK].rearrange(
                        "P (b N) -> P b N", b=4))
                nc.sync.dma_start(
                    x4[:, :, g * IBLK:(g + 1) * IBLK],
                    x8d[:, g * 4 * IBLK:(g + 1) * 4 * IBLK].rearrange(
                        "P (b N) -> P b N", b=4))

            q4r = q4f[:].rearrange("P (s N) -> P s N", s=2)
            k4r = k4f[:].rearrange("P (s N) -> P s N", s=2)
            wq4 = wq_sb[:].rearrange("P (b d) -> P b d", b=4)
            wk4 = wk_sb[:].rearrange("P (b d) -> P b d", b=4)
            wv4 = wv_sb[:].rearrange("P (b c) -> P b c", b=4)
            y64 = y_sb[0:64, :].rearrange("P (b N) -> P b N", b=4)
            ones_pair = ones_sb[:].rearrange("P (s c) -> P s c", s=2)

            def energy_at(n, jt0, nt, pt):
                # nt j-tiles of E^T[j, i-block n] starting at j-tile jt0 as
                # fp8 DoubleRow matmuls + one exp() into pt
                c0, c1 = n * IBLK, (n + 1) * IBLK
                et = psE.tile([128, 3 * IBLK], F32,
                              name=f"et_{n}_{jt0}", tag="et", bufs=2)
                for t in range(nt):
                    jt = jt0 + t
                    nc.tensor.matmul(
                        et[:, t * IBLK:(t + 1) * IBLK],
                        k4r[:, :, jt * 128:(jt + 1) * 128],
                        q4r[:, :, c0:c1],
                        start=True, stop=True,
                        perf_mode=DROW,
                    )
                nc.scalar.activation(
                    pt[:, jt0 * IBLK:(jt0 + nt) * IBLK],
                    et[:, 0:nt * IBLK], EXPF,
                )

            def energy(n, g, pt):
                # default grouping: 10x3 + 1x2 j-tiles
                nt = 3 if g < NGE - 1 else 2
                energy_at(n, 3 * g, nt, pt)

            def ptp_ap(pt, p):
                return pt[:, 2 * p * IBLK:(2 * p + 2) * IBLK].rearrange(
                    "P (s N) -> P s N", s=2)

            def den_pairs(pt, den, pairs):
                for p in pairs:
                    nc.tensor.matmul(
                        den[:], ones_pair, ptp_ap(pt, p),
                        start=(p == 0), stop=(p == NPAIR - 1),
                        perf_mode=DROW, skip_group_check=True,
                    )

            def av_pairs(pt, av, ch, pairs):
                for p in pairs:
                    nc.tensor.matmul(
                        av[:],
                        vt[:, 2 * p * C:(2 * p + 2) * C].rearrange(
                            "P (s c) -> P s c", s=2)[:, :, ch * 128:(ch + 1) * 128],
                        ptp_ap(pt, p),
                        start=(p == 0), stop=(p == NPAIR - 1),
                        perf_mode=DROW, skip_group_check=True,
                    )

            def den_tail(n, den):
                # rgb = 1 / den (gamma is folded into Wv/bv on the host)
                rgb = wrk.tile([128, IBLK], F32, name=f"rgb_{n}", tag="rgb",
                               bufs=3)
                nc.vector.reciprocal(rgb[:], den[:])
                return rgb

            def xf_fetch(n):
                xs = []
                for ch in range(CH):
                    xf_t = wrk.tile([128, IBLK], F32, name=f"xf_{n}_{ch}",
                                    tag="xf", bufs=6)
                    nc.gpsimd.dma_start(
                        xf_t[:],
                        xfd[ch * 128:(ch + 1) * 128, n * IBLK:(n + 1) * IBLK],
                    )
                    xs.append(xf_t)
                return xs

            def tail_ch(n, ch, av, rgb, xf_t, dma_engine=None):
                tmp = wrk.tile([128, IBLK], F32, name=f"tmp_{n}_{ch}", tag="tmp")
                nc.vector.tensor_tensor(tmp[:], av[:], rgb[:], MULT)
                ot = wrk.tile([128, IBLK], F32, name=f"ot_{n}_{ch}", tag="ot",
                              bufs=3)
                nc.vector.scalar_tensor_tensor(
                    ot[:], tmp[:], gbv_sb[:, ch:ch + 1], xf_t[:], ADD, ADD)
                (dma_engine or nc.sync).dma_start(
                    out[ch * 128:(ch + 1) * 128, n * IBLK:(n + 1) * IBLK], ot[:])

            pts = {}
            accs = {}
            rgbs = {}
            xfs = {}

            def new_block(n):
                pts[n] = ptp.tile([128, NJT * IBLK], F8, name=f"pt_{n}", tag="pt")

            # ------------- block 0: k/q projections + energy --------------
            with tc.tile_pool(name="psP1", bufs=1, space="PSUM") as psP1:
                def kq_proj(w4, src4, dstr, tag, sc, on_act=False,
                            use_et=False):
                    c0, c1 = sc * 256, (sc + 1) * 256
                    if use_et:
                        # the et ring is idle until the first energy matmul;
                        # borrowing slots gives the ramp-critical projections
                        # independent banks (no ring wait on the conversion)
                        ps = psE.tile([128, 3 * IBLK], F32, name=f"{tag}_{sc}",
                                      tag="et", bufs=2)[0:16, 0:512]
                    else:
                        ps = psP1.tile([16, 512], F32, name=f"{tag}_{sc}",
                                       tag="kq_ps", bufs=2)
                    ps3 = ps.rearrange("P (s N) -> P s N", s=2)
                    for s in range(2):
                        for h in range(CH):
                            nc.tensor.matmul(
                                ps3[:, s, :],
                                w4[:, 2 * h:2 * h + 2, 16 * s:16 * (s + 1)],
                                src4[:, 2 * h:2 * h + 2, c0:c1],
                                start=(h == 0), stop=(h == CH - 1),
                                perf_mode=DROW, skip_group_check=True)
                    if on_act:
                        # the Activation engine is idle until the first exp;
                        # converting the ramp-critical sub-chunks there runs
                        # in parallel with the DVE copies
                        nc.scalar.copy(dstr[:, :, c0:c1], ps3[:])
                    else:
                        nc.vector.tensor_copy(dstr[:, :, c0:c1], ps3[:])

                def k_proj(sc, on_act=False, use_et=False):
                    kq_proj(wk4, y4, k4r, "kps", sc, on_act, use_et)

                def q_proj(sc, on_act=False, use_et=False):
                    kq_proj(wq4, x4, q4r, "qps", sc, on_act, use_et)

                new_block(0)
                for g in range(NGE):
                    if g == 0:
                        k_proj(0, on_act=True, use_et=True)
                        k_proj(1)
                        q_proj(0, on_act=True, use_et=True)
                        q_proj(1)
                        k_proj(2, on_act=True)
                        for sc in range(3, 8):
                            k_proj(sc)
                    elif g <= 2:
                        # all remaining k first (energy consumes 1.5 k-subs
                        # per group); q sub-chunks 2n/2n+1 are only needed
                        # from block n onward, so they trail
                        for sc in range(8 + 4 * (g - 1), 8 + 4 * g):
                            k_proj(sc)
                    elif g <= 9:
                        q_proj(2 * g - 4)
                        q_proj(2 * g - 3)
                    energy(0, g, pts[0])

            # ------------- block 1: v projections + energy ----------------
            with tc.tile_pool(name="psP2", bufs=1, space="PSUM") as psP2:
                def v_proj(vp):
                    # one pv tile = 2 j-tiles
                    ps = psP2.tile([128, IBLK], F32, name=f"vps_{vp}",
                                   tag="pv_ps", bufs=2)
                    for t in range(2):
                        jt = 2 * vp + t
                        for h in range(CH):
                            nc.tensor.matmul(
                                ps[:, t * 256:(t + 1) * 256],
                                y64[:, 2 * h:2 * h + 2, jt * 128:(jt + 1) * 128],
                                wv4[:, 2 * h:2 * h + 2, :],
                                start=(h == 0), stop=(h == CH - 1),
                                perf_mode=DROW, skip_group_check=True)
                    nc.vector.tensor_copy(
                        vt[:, 2 * vp * C:(2 * vp + 2) * C], ps[:])

                new_block(1)
                for g in range(NGE):
                    energy(1, g, pts[1])
                    if g < 8:
                        v_proj(2 * g)
                        v_proj(2 * g + 1)
                xfs[0] = xf_fetch(0)

            # ---- blocks 2..7: deferred den/av bursts in a 2-bank ring ----
            with tc.tile_pool(name="psAV", bufs=1, space="PSUM") as psAV:
                def new_acc(name):
                    accs[name] = psAV.tile([128, IBLK], F32, name=name,
                                           tag="acc", bufs=2)
                    return accs[name]

                def den_burst(m):
                    d = new_acc(f"den_{m}")
                    den_pairs(pts[m], d, range(NPAIR))
                    rgbs[m] = den_tail(m, d)

                def av_burst(m, ch, dma_engine=None):
                    a = new_acc(f"av{ch}_{m}")
                    av_pairs(pts[m], a, ch, range(NPAIR))
                    tail_ch(m, ch, a, rgbs[m], xfs[m][ch], dma_engine)

                # block 2 carries the bursts of blocks 0 AND 1
                new_block(2)
                for g in range(NGE):
                    energy(2, g, pts[2])
                    if g == 0:
                        den_burst(0)
                        xfs[1] = xf_fetch(1)
                    elif g == 2:
                        av_burst(0, 0)
                    elif g == 4:
                        av_burst(0, 1)
                    elif g == 5:
                        den_burst(1)
                    elif g == 7:
                        av_burst(1, 0)
                    elif g == 9:
                        av_burst(1, 1)

                # blocks 3..6: steady state
                for n in range(3, NIB - 1):
                    new_block(n)
                    for g in range(NGE):
                        energy(n, g, pts[n])
                        if g == 0:
                            den_burst(n - 1)
                            xfs[n - 1] = xf_fetch(n - 1)
                        elif g == 2:
                            av_burst(n - 1, 0)
                        elif g == 4:
                            av_burst(n - 1, 1)

                # block 7: block 6's bursts early, then eager den(7)/av0(7)
                # in the freed ring slots; av1(7) accumulates in an et-ring
                # bank that frees after the second-to-last act, so only one
                # pair of each accumulator remains after the last exp()
                new_block(7)
                den7 = av07 = av17 = None
                issued_d = issued_a = 0
                for g in range(NGE):
                    energy(7, g, pts[7])
                    if g == 0:
                        den_burst(6)
                        xfs[6] = xf_fetch(6)
                    elif g == 2:
                        av_burst(6, 0)
                    elif g == 4:
                        av_burst(6, 1)
                    elif g == 6:
                        xfs[7] = xf_fetch(7)
                    if g >= 5:
                        if den7 is None:
                            den7 = new_acc("den_7")
                        hi = _pair_hi(g - 1)
                        den_pairs(pts[7], den7, range(issued_d, hi + 1))
                        issued_d = hi + 1
                    if g >= 7:
                        if av07 is None:
                            av07 = new_acc("av0_7")
                        hi = _pair_hi(g - 1)
                        av_pairs(pts[7], av07, 0, range(issued_a, hi + 1))
                        issued_a = hi + 1
                    if g == NGE - 1:
                        av17 = psE.tile([128, 3 * IBLK], F32, name="av1_7",
                                        tag="et", bufs=2)[:, 0:IBLK]
                        av_pairs(pts[7], av17, 1, range(_pair_hi(g - 1) + 1))

                # drain: den/av0 flushes + reciprocal run at high priority so
                # the PE/DVE prefer them over av17's bulk pairs
                with tc.high_priority():
                    den_pairs(pts[7], den7, range(issued_d, NPAIR))
                    rgbs[7] = den_tail(7, den7)
                    av_pairs(pts[7], av07, 0, range(issued_a, NPAIR))
                av_pairs(pts[7], av17, 1, range(_pair_hi(NGE - 2) + 1, NPAIR))
                tail_ch(7, 0, av07, rgbs[7], xfs[7][0])
                tail_ch(7, 1, av17, rgbs[7], xfs[7][1])
    nc.compile()
    return nc


_NC_CACHE = {}


def _fold65(a):
    # [C=256, HW-like cols] -> [64, (chunk, h, s, blk)] fp8 with channel
    # c = h*128 + s*64 + p, chunk-major columns
    cols = a.shape[1]
    nb = cols // IBLK
    return (a.reshape(2, 2, 64, nb, IBLK).transpose(2, 3, 0, 1, 4)
            .reshape(64, 4 * cols))


def kernel(x, y, Wq, bq, Wk, bk, Wv, bv, gamma):
    assert x.shape == (B, C, 64, 64)
    xs = np.ascontiguousarray(x.reshape(B, C, HW)).astype(np.float32)
    ys = np.ascontiguousarray(y.reshape(B, C, HW)).astype(np.float32)
    g = float(np.asarray(gamma).reshape(-1)[0])

    ones_row = np.ones((1, 4 * HW), dtype=np.float32)
    x8 = np.stack([
        np.concatenate([_fold65(xs[b]), ones_row], axis=0) for b in range(B)
    ]).astype(F8NP)
    y8 = np.stack([
        np.concatenate([_fold65(ys[b]), ones_row], axis=0) for b in range(B)
    ]).astype(F8NP)

    # weights: [64, (h, s, d)] body + bias row (bias lives in the (h=0, s)
    # bands; the ones-row of x/y multiplies it once)
    def _wfold(w, bias):
        body = (w.T.reshape(2, 2, 64, D).transpose(2, 0, 1, 3)
                .reshape(64, 4 * D))
        brow = np.zeros((1, 4 * D), dtype=np.float32)
        brow[0, 0:D] = bias
        return np.concatenate([body, brow], axis=0)

    wq8 = _wfold(Wq.astype(np.float32), bq.astype(np.float32)).astype(F8NP)
    wk8 = _wfold(Wk.astype(np.float32), bk.astype(np.float32)).astype(F8NP)
    wv8 = ((g * Wv).T.reshape(2, 2, 64, C).transpose(2, 0, 1, 3)
           .reshape(64, 4 * C).astype(F8NP))
    gbvh = np.ascontiguousarray((g * bv.astype(np.float32)).reshape(CH, 128).T)

    if "nc" not in _NC_CACHE:
        _NC_CACHE["nc"] = _build()
    nc = _NC_CACHE["nc"]

    in_maps = [
        {
            "x8d": np.ascontiguousarray(x8[b]),
            "y8d": np.ascontiguousarray(y8[b]),
            "xfd": np.ascontiguousarray(xs[b]),
            "wq8d": np.ascontiguousarray(wq8),
            "wk8d": np.ascontiguousarray(wk8),
            "wv8d": np.ascontiguousarray(wv8),
            "gbvd": gbvh,
        }
        for b in range(B)
    ]
    res = run_bass_kernel_spmd(nc, in_maps, list(range(B)))
    outs = np.stack([res.results[b]["out"] for b in range(B)])
    return outs.reshape(B, C, 64, 64).astype(np.float32)


# revision 47
# speedup vs baseline: 1.0088x; 1.0088x over previous
"""Cross-modal attention kernel for Trainium2 -- data-parallel over batch on 8 cores.

Reference computation per sample (C=256, H=W=64, N=H*W=4096, dqk=32):
    q = Wq @ x + bq; k = Wk @ y + bk; v = Wv @ y + bv
    out = gamma * (v @ softmax_j(q^T k)^T) + x

The Activation engine is the roofline: exp() over the [N, N] energy matrix is
131072 columns at 0.8333 ns/col, plus a fixed ~185 ns access penalty per
activation instruction.  The schedule maximizes exp() instruction size
([128, 1536] = 3 PSUM banks, 11 instructions per i-block, ~127 us busy) and
keeps the engine streaming gap-free from ~6 us on:

  - Projections are FUSED into the first two i-blocks: all k/q sub-chunks
    run just ahead of block 0's energy matmuls, v during block 1.
  - q/k are built directly in the [16, 2, N] DoubleRow layout: each 256-col
    sub-chunk is four fp8 DoubleRow matmuls into a [16, 2, 256] PSUM tile and
    one 3D-AP DVE tensor_copy (f32 -> fp8).  The bias rides along as a 65th
    contraction row (x/y carry a ones-row, Wq/Wk carry the bias).
  - Inputs are uploaded pre-quantized fp8 (2 MB total): gamma is folded into
    Wv/bv so the softmax scale step disappears; the f32 residual x streams in
    during the steady state when DMA is idle.
  - Every matmul runs fp8 DoubleRow (0.5 cyc/row).  PE busy ~66 us.
  - PSUM (8 banks): et double-buffer 6 + one 2-bank ring that time-shares
    k/q projection tiles (block 0), v tiles (block 1), then den/AV:
    den and AV run as deferred 16-step DoubleRow bursts one block later
    (pt lives in a 3-deep SBUF ring), except the last block where den and
    AV-ch0 run eagerly in the freed ring slots so the post-exp drain is only
    the AV-ch1 burst plus one tail chain.
  - Softmax normalization happens on the [C, IBLK] output (reciprocal of the
    ones-matmul denominator), never on the [N, N] matrix.
"""

import sys

if "/opt/trn_rl_repo" not in sys.path:
    sys.path.insert(0, "/opt/trn_rl_repo")

import ml_dtypes
import numpy as np

import concourse.bacc as bacc
import concourse.mybir as mybir
import concourse.tile as tile
from concourse.bass_utils import run_bass_kernel_spmd

F32 = mybir.dt.float32
F8 = mybir.dt.float8e4
F8NP = ml_dtypes.float8_e4m3

B, C, HW, D = 8, 256, 4096, 32
CH = C // 128
IBLK = 512
NIB = HW // IBLK          # 8 i-blocks
NJT = HW // 128           # 32 j-tiles
NPAIR = NJT // 2          # 16 DoubleRow pairs
NGE = 11                  # energy groups per block: 10x3 + 1x2 j-tiles
EXPF = mybir.ActivationFunctionType.Exp
MULT = mybir.AluOpType.mult
ADD = mybir.AluOpType.add
DROW = mybir.MatmulPerfMode.DoubleRow


def _pair_hi(g):
    # highest den/av pair index ready after act group g of a block
    return min((3 * g + 1) // 2, NPAIR - 1)


def _build():
    nc = bacc.Bacc("TRN2", target_bir_lowering=False, debug=False, num_devices=8)

    x8d = nc.dram_tensor("x8d", [65, 4 * HW], F8, kind="ExternalInput")
    y8d = nc.dram_tensor("y8d", [65, 4 * HW], F8, kind="ExternalInput")
    xfd = nc.dram_tensor("xfd", [C, HW], F32, kind="ExternalInput")
    wq8d = nc.dram_tensor("wq8d", [65, 4 * D], F8, kind="ExternalInput")
    wk8d = nc.dram_tensor("wk8d", [65, 4 * D], F8, kind="ExternalInput")
    wv8d = nc.dram_tensor("wv8d", [64, 4 * C], F8, kind="ExternalInput")
    gbvd = nc.dram_tensor("gbvd", [128, CH], F32, kind="ExternalInput")
    out = nc.dram_tensor("out", [C, HW], F32, kind="ExternalOutput")

    tc = tile.TileContext(nc)
    with tc:
        with (
            tc.tile_pool(name="cst", bufs=1) as cst,
            tc.tile_pool(name="ptp", bufs=3) as ptp,
            tc.tile_pool(name="wrk", bufs=2) as wrk,
            tc.tile_pool(name="psE", bufs=1, space="PSUM") as psE,
        ):
            wq_sb = cst.tile([65, 4 * D], F8)
            wk_sb = cst.tile([65, 4 * D], F8)
            wv_sb = cst.tile([64, 4 * C], F8)
            gbv_sb = cst.tile([128, CH], F32)
            ones_sb = cst.tile([128, 2 * 128], F8)
            x_sb = cst.tile([65, 4 * HW], F8)
            y_sb = cst.tile([65, 4 * HW], F8)
            q4f = cst.tile([16, 2 * HW], F8)
            k4f = cst.tile([16, 2 * HW], F8)
            vt = cst.tile([128, NJT * C], F8)

            nc.vector.memset(ones_sb[:], 1.0)
            # small weights ride the SWDGE (gpsimd) queue, in parallel with
            # the HWDGE input stream; gbv isn't needed until block 2
            nc.gpsimd.dma_start(wk_sb[:], wk8d[:])
            nc.gpsimd.dma_start(wq_sb[:], wq8d[:])
            nc.gpsimd.dma_start(gbv_sb[:], gbvd[:])

            # inputs on the SP queue; k-path (y) leads
            y4 = y_sb[:].rearrange("P (b N) -> P b N", b=4)
            x4 = x_sb[:].rearrange("P (b N) -> P b N", b=4)
            nc.sync.dma_start(
                y4[:, :, 0:IBLK],
                y8d[:, 0:4 * IBLK].rearrange("P (b N) -> P b N", b=4))
            nc.sync.dma_start(
                x4[:, :, 0:IBLK],
                x8d[:, 0:4 * IBLK].rearrange("P (b N) -> P b N", b=4))
            nc.sync.dma_start(wv_sb[:], wv8d[:])
            for g in range(1, NIB):
                nc.sync.dma_start(
                    y4[:, :, g * IBLK:(g + 1) * IBLK],
                    y8d[:, g * 4 * IBLK:(g + 1) * 4 * IBLK].rearrange(
                        "P (b N) -> P b N", b=4))
                nc.sync.dma_start(
                    x4[:, :, g * IBLK:(g + 1) * IBLK],
                    x8d[:, g * 4 * IBLK:(g + 1) * 4 * IBLK].rearrange(
                        "P (b N) -> P b N", b=4))

            q4r = q4f[:].rearrange("P (s N) -> P s N", s=2)
            k4r = k4f[:].rearrange("P (s N) -> P s N", s=2)
            wq4 = wq_sb[:].rearrange("P (b d) -> P b d", b=4)
            wk4 = wk_sb[:].rearrange("P (b d) -> P b d", b=4)
            wv4 = wv_sb[:].rearrange("P (b c) -> P b c", b=4)
            y64 = y_sb[0:64, :].rearrange("P (b N) -> P b N", b=4)
            ones_pair = ones_sb[:].rearrange("P (s c) -> P s c", s=2)

            def energy_at(n, jt0, nt, pt):
                # nt j-tiles of E^T[j, i-block n] starting at j-tile jt0 as
                # fp8 DoubleRow matmuls + one exp() into pt
                c0, c1 = n * IBLK, (n + 1) * IBLK
                et = psE.tile([128, 3 * IBLK], F32,
                              name=f"et_{n}_{jt0}", tag="et", bufs=2)
                for t in range(nt):
                    jt = jt0 + t
                    nc.tensor.matmul(
                        et[:, t * IBLK:(t + 1) * IBLK],
                        k4r[:, :, jt * 128:(jt + 1) * 128],
                        q4r[:, :, c0:c1],
                        start=True, stop=True,
                        perf_mode=DROW,
                    )
                nc.scalar.activation(
                    pt[:, jt0 * IBLK:(jt0 + nt) * IBLK],
                    et[:, 0:nt * IBLK], EXPF,
                )

            def energy(n, g, pt):
                # default grouping: 10x3 + 1x2 j-tiles
                nt = 3 if g < NGE - 1 else 2
                energy_at(n, 3 * g, nt, pt)

            def ptp_ap(pt, p):
                return pt[:, 2 * p * IBLK:(2 * p + 2) * IBLK].rearrange(
                    "P (s N) -> P s N", s=2)

            def den_pairs(pt, den, pairs):
                for p in pairs:
                    nc.tensor.matmul(
                        den[:], ones_pair, ptp_ap(pt, p),
                        start=(p == 0), stop=(p == NPAIR - 1),
                        perf_mode=DROW, skip_group_check=True,
                    )

            def av_pairs(pt, av, ch, pairs):
                for p in pairs:
                    nc.tensor.matmul(
                        av[:],
                        vt[:, 2 * p * C:(2 * p + 2) * C].rearrange(
                            "P (s c) -> P s c", s=2)[:, :, ch * 128:(ch + 1) * 128],
                        ptp_ap(pt, p),
                        start=(p == 0), stop=(p == NPAIR - 1),
                        perf_mode=DROW, skip_group_check=True,
                    )

            def den_tail(n, den):
                # rgb = 1 / den (gamma is folded into Wv/bv on the host)
                rgb = wrk.tile([128, IBLK], F32, name=f"rgb_{n}", tag="rgb",
                               bufs=3)
                nc.vector.reciprocal(rgb[:], den[:])
                return rgb

            def xf_fetch(n):
                xs = []
                for ch in range(CH):
                    xf_t = wrk.tile([128, IBLK], F32, name=f"xf_{n}_{ch}",
                                    tag="xf", bufs=6)
                    nc.gpsimd.dma_start(
                        xf_t[:],
                        xfd[ch * 128:(ch + 1) * 128, n * IBLK:(n + 1) * IBLK],
                    )
                    xs.append(xf_t)
                return xs

            def tail_ch(n, ch, av, rgb, xf_t, dma_engine=None):
                tmp = wrk.tile([128, IBLK], F32, name=f"tmp_{n}_{ch}", tag="tmp")
                nc.vector.tensor_tensor(tmp[:], av[:], rgb[:], MULT)
                ot = wrk.tile([128, IBLK], F32, name=f"ot_{n}_{ch}", tag="ot",
                              bufs=3)
                nc.vector.scalar_tensor_tensor(
                    ot[:], tmp[:], gbv_sb[:, ch:ch + 1], xf_t[:], ADD, ADD)
                (dma_engine or nc.sync).dma_start(
                    out[ch * 128:(ch + 1) * 128, n * IBLK:(n + 1) * IBLK], ot[:])

            pts = {}
            accs = {}
            rgbs = {}
            xfs = {}

            def new_block(n):
                pts[n] = ptp.tile([128, NJT * IBLK], F8, name=f"pt_{n}", tag="pt")

            # ------------- block 0: k/q projections + energy --------------
            with tc.tile_pool(name="psP1", bufs=1, space="PSUM") as psP1:
                def kq_proj(w4, src4, dstr, tag, sc, on_act=False,
                            use_et=False):
                    c0, c1 = sc * 256, (sc + 1) * 256
                    if use_et:
                        # the et ring is idle until the first energy matmul;
                        # borrowing slots gives the ramp-critical projections
                        # independent banks (no ring wait on the conversion)
                        ps = psE.tile([128, 3 * IBLK], F32, name=f"{tag}_{sc}",
                                      tag="et", bufs=2)[0:16, 0:512]
                    else:
                        ps = psP1.tile([16, 512], F32, name=f"{tag}_{sc}",
                                       tag="kq_ps", bufs=2)
                    ps3 = ps.rearrange("P (s N) -> P s N", s=2)
                    for s in range(2):
                        for h in range(CH):
                            nc.tensor.matmul(
                                ps3[:, s, :],
                                w4[:, 2 * h:2 * h + 2, 16 * s:16 * (s + 1)],
                                src4[:, 2 * h:2 * h + 2, c0:c1],
                                start=(h == 0), stop=(h == CH - 1),
                                perf_mode=DROW, skip_group_check=True)
                    if on_act:
                        # the Activation engine is idle until the first exp;
                        # converting the ramp-critical sub-chunks there runs
                        # in parallel with the DVE copies
                        nc.scalar.copy(dstr[:, :, c0:c1], ps3[:])
                    else:
                        nc.vector.tensor_copy(dstr[:, :, c0:c1], ps3[:])

                def k_proj(sc, on_act=False, use_et=False):
                    kq_proj(wk4, y4, k4r, "kps", sc, on_act, use_et)

                def q_proj(sc, on_act=False, use_et=False):
                    kq_proj(wq4, x4, q4r, "qps", sc, on_act, use_et)

                new_block(0)
                for g in range(NGE):
                    if g == 0:
                        k_proj(0, on_act=True, use_et=True)
                        k_proj(1)
                        q_proj(0, on_act=True, use_et=True)
                        q_proj(1)
                        k_proj(2, on_act=True)
                        for sc in range(3, 8):
                            k_proj(sc)
                    elif g <= 2:
                        # all remaining k first (energy consumes 1.5 k-subs
                        # per group); q sub-chunks 2n/2n+1 are only needed
                        # from block n onward, so they trail
                        for sc in range(8 + 4 * (g - 1), 8 + 4 * g):
                            k_proj(sc)
                    elif g <= 9:
                        q_proj(2 * g - 4)
                        q_proj(2 * g - 3)
                    energy(0, g, pts[0])

            # ------------- block 1: v projections + energy ----------------
            with tc.tile_pool(name="psP2", bufs=1, space="PSUM") as psP2:
                def v_proj(vp):
                    # one pv tile = 2 j-tiles
                    ps = psP2.tile([128, IBLK], F32, name=f"vps_{vp}",
                                   tag="pv_ps", bufs=2)
                    for t in range(2):
                        jt = 2 * vp + t
                        for h in range(CH):
                            nc.tensor.matmul(
                                ps[:, t * 256:(t + 1) * 256],
                                y64[:, 2 * h:2 * h + 2, jt * 128:(jt + 1) * 128],
                                wv4[:, 2 * h:2 * h + 2, :],
                                start=(h == 0), stop=(h == CH - 1),
                                perf_mode=DROW, skip_group_check=True)
                    nc.vector.tensor_copy(
                        vt[:, 2 * vp * C:(2 * vp + 2) * C], ps[:])

                new_block(1)
                for g in range(NGE):
                    energy(1, g, pts[1])
                    if g < 8:
                        v_proj(2 * g)
                        v_proj(2 * g + 1)
                xfs[0] = xf_fetch(0)

            # ---- blocks 2..7: deferred den/av bursts in a 2-bank ring ----
            with tc.tile_pool(name="psAV", bufs=1, space="PSUM") as psAV:
                def new_acc(name):
                    accs[name] = psAV.tile([128, IBLK], F32, name=name,
                                           tag="acc", bufs=2)
                    return accs[name]

                def den_burst(m):
                    d = new_acc(f"den_{m}")
                    den_pairs(pts[m], d, range(NPAIR))
                    rgbs[m] = den_tail(m, d)

                def av_burst(m, ch, dma_engine=None):
                    a = new_acc(f"av{ch}_{m}")
                    av_pairs(pts[m], a, ch, range(NPAIR))
                    tail_ch(m, ch, a, rgbs[m], xfs[m][ch], dma_engine)

                # block 2 carries the bursts of blocks 0 AND 1
                new_block(2)
                for g in range(NGE):
                    energy(2, g, pts[2])
                    if g == 0:
                        den_burst(0)
                        xfs[1] = xf_fetch(1)
                    elif g == 2:
                        av_burst(0, 0)
                    elif g == 4:
                        av_burst(0, 1)
                    elif g == 5:
                        den_burst(1)
                    elif g == 7:
                        av_burst(1, 0)
                    elif g == 9:
                        av_burst(1, 1)

                # blocks 3..6: steady state
                for n in range(3, NIB - 1):
                    new_block(n)
                    for g in range(NGE):
                        energy(n, g, pts[n])
                        if g == 0:
                            den_burst(n - 1)
                            xfs[n - 1] = xf_fetch(n - 1)
                        elif g == 2:
                            av_burst(n - 1, 0)
                        elif g == 4:
                            av_burst(n - 1, 1)

                # block 7: block 6's bursts early, then eager den(7)/av0(7)
                # in the freed ring slots; av1(7) accumulates in an et-ring
                # bank that frees after the second-to-last act, so only one
                # pair of each accumulator remains after the last exp()
                new_block(7)
                den7 = av07 = av17 = None
                issued_d = issued_a = 0
                for g in range(NGE):
                    energy(7, g, pts[7])
                    if g == 0:
                        den_burst(6)
                        xfs[6] = xf_fetch(6)
                    elif g == 2:
                        av_burst(6, 0)
                    elif g == 4:
                        av_burst(6, 1)
                    elif g == 6:
                        xfs[7] = xf_fetch(7)
                    if g >= 5:
                        if den7 is None:
                            den7 = new_acc("den_7")
                        hi = _pair_hi(g - 1)
                        den_pairs(pts[7], den7, range(issued_d, hi + 1))
                        issued_d = hi + 1
                    if g >= 7:
                        if av07 is None:
                            av07 = new_acc("av0_7")
                        hi = _pair_hi(g - 1)
                        av_pairs(pts[7], av07, 0, range(issued_a, hi + 1))
                        issued_a = hi + 1
                    if g == NGE - 1:
                        av17 = psE.tile([128, 3 * IBLK], F32, name="av1_7",
                                        tag="et", bufs=2)[:, 0:IBLK]
                        av_pairs(pts[7], av17, 1, range(_pair_hi(g - 1) + 1))

                # drain: den/av0 flushes + reciprocal run at high priority so
                # the PE/DVE prefer them over av17's bulk pairs
                with tc.high_priority():
                    den_pairs(pts[7], den7, range(issued_d, NPAIR))
                    rgbs[7] = den_tail(7, den7)
                    av_pairs(pts[7], av07, 0, range(issued_a, NPAIR))
                av_pairs(pts[7], av17, 1, range(_pair_hi(NGE - 2) + 1, NPAIR))
                tail_ch(7, 0, av07, rgbs[7], xfs[7][0])
                tail_ch(7, 1, av17, rgbs[7], xfs[7][1])
    nc.compile()
    return nc


_NC_CACHE = {}


def _fold65(a):
    # [C=256, HW-like cols] -> [64, (chunk, h, s, blk)] fp8 with channel
    # c = h*128 + s*64 + p, chunk-major columns
    cols = a.shape[1]
    nb = cols // IBLK
    return (a.reshape(2, 2, 64, nb, IBLK).transpose(2, 3, 0, 1, 4)
            .reshape(64, 4 * cols))


def kernel(x, y, Wq, bq, Wk, bk, Wv, bv, gamma):
    assert x.shape == (B, C, 64, 64)
    xs = np.ascontiguousarray(x.reshape(B, C, HW)).astype(np.float32)
    ys = np.ascontiguousarray(y.reshape(B, C, HW)).astype(np.float32)
    g = float(np.asarray(gamma).reshape(-1)[0])

    ones_row = np.ones((1, 4 * HW), dtype=np.float32)
    x8 = np.stack([
        np.concatenate([_fold65(xs[b]), ones_row], axis=0) for b in range(B)
    ]).astype(F8NP)
    y8 = np.stack([
        np.concatenate([_fold65(ys[b]), ones_row], axis=0) for b in range(B)
    ]).astype(F8NP)

    # weights: [64, (h, s, d)] body + bias row (bias lives in the (h=0, s)
    # bands; the ones-row of x/y multiplies it once)
    def _wfold(w, bias):
        body = (w.T.reshape(2, 2, 64, D).transpose(2, 0, 1, 3)
                .reshape(64, 4 * D))
        brow = np.zeros((1, 4 * D), dtype=np.float32)
        brow[0, 0:D] = bias
        return np.concatenate([body, brow], axis=0)

    wq8 = _wfold(Wq.astype(np.float32), bq.astype(np.float32)).astype(F8NP)
    wk8 = _wfold(Wk.astype(np.float32), bk.astype(np.float32)).astype(F8NP)
    wv8 = ((g * Wv).T.reshape(2, 2, 64, C).transpose(2, 0, 1, 3)
           .reshape(64, 4 * C).astype(F8NP))
    gbvh = np.ascontiguousarray((g * bv.astype(np.float32)).reshape(CH, 128).T)

    if "nc" not in _NC_CACHE:
        _NC_CACHE["nc"] = _build()
    nc = _NC_CACHE["nc"]

    in_maps = [
        {
            "x8d": np.ascontiguousarray(x8[b]),
            "y8d": np.ascontiguousarray(y8[b]),
            "xfd": np.ascontiguousarray(xs[b]),
            "wq8d": np.ascontiguousarray(wq8),
            "wk8d": np.ascontiguousarray(wk8),
            "wv8d": np.ascontiguousarray(wv8),
            "gbvd": gbvh,
        }
        for b in range(B)
    ]
    res = run_bass_kernel_spmd(nc, in_maps, list(range(B)))
    outs = np.stack([res.results[b]["out"] for b in range(B)])
    return outs.reshape(B, C, 64, 64).astype(np.float32)
